# revision 39
# baseline (speedup 1.0000x reference)
"""Tensor-parallel attention block (QKV + RoPE + causal attention + out-proj)
for 8 Trainium2 NeuronCores.

Sharding: heads (16) split across 8 cores, 2 heads/core. wq/wk/wv column-
sharded, wo row-sharded; each core computes a full-shape partial output (bf16)
and the host sums the 8 partials in f32.

Measured on this part: PE runs at 2.0 GHz warm / 1.0 GHz cold (HAM), and the
kernel is PE-streaming-cycle bound, so the design minimizes PE cycles and
keeps the PE stream dense:
  - softmax denominator off the PE: the DVE accumulates exp chunks into
    two interleaved bf16 accumulators; four ones-matmuls per (b,qw) group
    broadcast the column sums; reciprocal via exp(-ln(d)) on ACT (exact
    DVE reciprocal is 8 cyc/elem, ACT Reciprocal LUT is banned for
    accuracy). GpSimd is kept off elementwise work: it shares SBUF ports
    with the DVE and starves it.
  - causal masking folded into the scores PSUM as an extra accumulate
    matmul (lhsT=identity, rhs=-30000*pattern): exp then yields exact
    zeros — no elementwise mask multiply, no extra engine hop between
    exp and the AV matmul.
  - dead causal regions skipped at column granularity (scores/exp/AV only
    over q >= off per (qw, kc) tile, off derived from the real mask).
  - V projection emitted directly in [seq, hd] layout (stationary = x
    chunk): no PE transposes.
  - out-projection pipelined one group late and its matmuls woven through
    the next group's chunk loop from a shared 3-deep PSUM ring
    (psBig 6 banks + psAtt 2 banks = all 8), so PE always has independent
    work while ACT exp latency drains; the raw attention sums are copied
    out of PSUM immediately (normalization happens off the critical path)
    so the single att accumulator recycles without waiting on the
    reciprocal chain.
  - RoPE is applied per seq-half right after that half's q/k projection
    copies, so it runs on an otherwise-idle DVE during QKV matmuls and
    the attention phase never waits on it.
  - both heads packed side by side in [128, 2, 512] tiles: one ACT exp per
    chunk, one reciprocal chain per group.
  - output written bf16; out DMAs are [128, 2048] on the SP (sync) hwdge
    queue; weight loads also on SP to parallelize with the x loads on the
    gpsimd queue.
"""

import math
import os
import sys

import numpy as np
import ml_dtypes

sys.path.insert(0, "/opt/trn_rl_repo")

import concourse.bass as bass
import concourse.mybir as mybir
from concourse.tile import TileContext
from concourse.bass_utils import run_bass_kernel_spmd
from concourse.masks import make_identity

B, S, D, H = 2, 2048, 2048, 16
HD = D // H            # 128 head dim
NCORES = 8
HC = H // NCORES       # 2 heads per core
DHC = HC * HD          # 256
BS = B * S             # 4096
NDIN = D // 128        # 16 contraction chunks
W = 512                # attention q-window
NQW = S // W           # 4 q windows per batch
NKC = S // 128         # 16 k chunks per batch
SG = 1024              # qkv seq-group width
NSG = BS // SG         # 4
RSQRT_HD = 1.0 / math.sqrt(HD)
NEGBIG = -30000.0      # additive mask value; * RSQRT_HD still << exp underflow

BF16 = mybir.dt.bfloat16
F32 = mybir.dt.float32
NPBF16 = ml_dtypes.bfloat16

SKIP, FREE, MASKED = 0, 1, 2

# stash of the last BassKernelResults for the test harness (exec_time_ns etc)
LAST_RUN = [None]
_PROGRAM_CACHE = {}


def _split_multi_waits(nc):
    """Walrus codegen allows only 1 embedded sync-wait per instruction (2 for
    EventSemaphore). Tile's sem-assignment can emit more; hoist the excess into
    standalone InstEventSemaphore waits on the same engine, just before."""
    n = 0
    for blk in nc.m.functions[0].blocks:
        out = []
        for inst in blk.instructions:
            si = getattr(inst, "sync_info", None)
            cap = 2 if isinstance(inst, mybir.InstEventSemaphore) else 1
            if si is not None and si.on_wait and len(si.on_wait) > cap:
                waits = list(si.on_wait)
                for w in waits[:-cap]:
                    n += 1
                    ev = mybir.InstEventSemaphore(
                        name=f"{inst.name}_xw{n}",
                        ins=[], outs=[],
                        sync_info=mybir.SyncInfo(on_wait=[w], on_update=[]))
                    ev.engine = inst.engine
                    out.append(ev)
                si.on_wait = waits[-cap:]
            out.append(inst)
        blk.instructions = out


def _build(cls_key, n_pm):
    """Build the per-core Bass program.

    cls_key: tuple[NQW][NKC] of (kind, off, pm_id, p0, p1); n_pm: number of
    unique deduped [128, W] additive-mask pattern blocks in `pmask`."""
    cls = [list(row) for row in cls_key]
    nc = bass.Bass()

    xT = nc.declare_dram_parameter("xT", [D, BS], BF16, isOutput=False)
    wqT = nc.declare_dram_parameter("wqT", [D, DHC], BF16, isOutput=False)
    wkT = nc.declare_dram_parameter("wkT", [D, DHC], BF16, isOutput=False)
    wvT = nc.declare_dram_parameter("wvT", [D, DHC], BF16, isOutput=False)
    woT = nc.declare_dram_parameter("woT", [DHC, D], BF16, isOutput=False)
    trig = nc.declare_dram_parameter("trig", [128, 2 * S], BF16, isOutput=False)
    pmask = nc.declare_dram_parameter("pmask", [128, max(n_pm, 1), W], BF16,
                                      isOutput=False)
    out_d = nc.declare_dram_parameter("out", [BS, D], BF16, isOutput=True)

    with TileContext(nc) as tc:
        with (
            tc.tile_pool(name="consts", bufs=1) as consts,
            tc.tile_pool(name="xt", bufs=7) as xtp,
            tc.tile_pool(name="rsw", bufs=2) as rswp,
            tc.tile_pool(name="rtmp", bufs=2) as rtp,
            tc.tile_pool(name="pb", bufs=6) as pbp,
            tc.tile_pool(name="acc", bufs=4) as accp,
            tc.tile_pool(name="apair", bufs=4) as app,
            tc.tile_pool(name="rc", bufs=2) as rcp,
            tc.tile_pool(name="ob", bufs=2) as obp,
            tc.tile_pool(name="psBig", bufs=3, space="PSUM") as psBig,
            tc.tile_pool(name="psAtt", bufs=1, space="PSUM") as psAtt,
        ):
            # ---- persistent tiles ----
            # per-batch q/k tiles: rope for batch 1 must not create (false)
            # write-dependencies against batch-0 attention reads
            q_sb = [[consts.tile([128, S], BF16, tag=f"q{b}{h}", name=f"q{b}{h}")
                     for h in range(HC)] for b in range(B)]
            k_sb = [[consts.tile([128, S], BF16, tag=f"k{b}{h}", name=f"k{b}{h}")
                     for h in range(HC)] for b in range(B)]
            vT_sb = consts.tile([128, B * NKC * DHC], BF16, tag="vT", name="vT")
            ones = consts.tile([128, 128], BF16, tag="ones", name="ones")
            nc.vector.memset(ones, 1.0)
            ident = consts.tile([128, 128], BF16, tag="ident", name="ident")
            make_identity(nc, ident)

            # warm the PE clock (HAM releases the throttle after ~3.4us of
            # sustained activity) while the first DMAs are in flight
            wu = psAtt.tile([128, 128], F32, tag="psA", name="warmup")
            for i in range(190):
                nc.tensor.matmul(wu, lhsT=ones, rhs=ones, start=True, stop=True)

            # weights resident: [128, NDIN, DHC], halves loaded separately so
            # the first projection matmuls can start sooner
            w_all = []
            for wi_, wparam in enumerate([wqT, wkT, wvT]):
                wt_ = consts.tile([128, NDIN, DHC], BF16, tag=f"wall{wi_}", name=f"wall{wi_}")
                # wq split finer: the first projection matmuls need only di 0-3
                bounds = [0, 4, 8, 16] if wi_ == 0 else [0, 8, 16]
                for lo, hi in zip(bounds, bounds[1:]):
                    nc.sync.dma_start(
                        out=wt_[:, lo:hi, :],
                        in_=wparam[lo * 128:hi * 128, :].rearrange(
                            "(n p) m -> p n m", p=128))
                w_all.append(wt_)
            trig_sb = consts.tile([128, 2 * S], BF16, tag="trig", name="trig")
            nc.sync.dma_start(out=trig_sb, in_=trig[:, :])
            woT_sb = [consts.tile([128, D], BF16, tag=f"wo{h}", name=f"wo{h}")
                      for h in range(HC)]
            for h in range(HC):
                nc.sync.dma_start(out=woT_sb[h], in_=woT[h * 128:(h + 1) * 128, :])
            pm_sb = []
            for e in range(n_pm):
                pmt = consts.tile([128, W], BF16, tag=f"pm{e}", name=f"pm{e}")
                nc.sync.dma_start(out=pmt, in_=pmask[:, e, :])
                pm_sb.append(pmt)

            def rope_half(b, tens, h, lo):
                """RoPE on positions [lo, lo+SG) of one head's q or k —
                emitted right after that seq-group's projection copies, so
                the DVE does it while the PE is still on QKV matmuls and the
                attention phase never waits on rope."""
                src = tens[b][h]
                hi = lo + SG
                sw = rswp.tile([128, SG], BF16, tag="rsw", name=f"sw{b}_{h}_{lo}")
                nc.scalar.dma_start(out=sw[0:64, :], in_=src[64:128, lo:hi])
                nc.scalar.dma_start(out=sw[64:128, :], in_=src[0:64, lo:hi])
                mcc = rtp.tile([128, SG], BF16, tag="mcc", name=f"mcc{b}_{h}_{lo}")
                nc.vector.tensor_mul(mcc, src[:, lo:hi], trig_sb[:, lo:hi])
                nc.vector.tensor_mul(sw, sw, trig_sb[:, S + lo:S + hi])
                nc.vector.tensor_add(src[:, lo:hi], mcc, sw)

            # ---- QKV projections (per batch) ----
            for b in range(B):
                for sg in range(2 * b, 2 * b + 2):
                    xts = []
                    for dj in range(4):
                        tb = xtp.tile([128, 4, SG], BF16, tag="xt", name=f"xt{sg}_{dj}")
                        if sg == 0:
                            for k4 in range(4):
                                r0 = dj * 512 + k4 * 128
                                nc.gpsimd.dma_start(
                                    out=tb[:, k4, :],
                                    in_=xT[r0:r0 + 128, 0:SG])
                        else:
                            nc.gpsimd.dma_start(
                                out=tb,
                                in_=xT[dj * 512:(dj + 1) * 512,
                                       sg * SG:(sg + 1) * SG].rearrange("(n p) m -> p n m", p=128))
                        for k4 in range(4):
                            xts.append(tb[:, k4, :])
                    # q, k: stationary = weight tile, moving = x
                    for ti in range(2):
                        for dh in range(HC):
                            ps = psBig.tile([128, 2, W], F32, tag="psBig",
                                            name=f"ps{sg}_{ti}_{dh}")
                            for di in range(NDIN):
                                for wi in range(2):
                                    nc.tensor.matmul(
                                        ps[:, wi, :],
                                        lhsT=w_all[ti][:, di, dh * 128:(dh + 1) * 128],
                                        rhs=xts[di][:, wi * W:(wi + 1) * W],
                                        start=(di == 0), stop=(di == NDIN - 1))
                            dst = (q_sb if ti == 0 else k_sb)[b][dh]
                            c0 = (sg % 2) * SG
                            with tc.high_priority():
                                nc.scalar.copy(
                                    dst[:, c0:c0 + SG].rearrange(
                                        "p (n m) -> p n m", n=2),
                                    ps)
                    # v: stationary = x chunk, moving = wv cols -> [seq, hd]
                    for sc in range(SG // 128):
                        vps = psBig.tile([128, DHC], F32, tag="psBig",
                                         name=f"vps{sg}_{sc}")
                        for di in range(NDIN):
                            nc.tensor.matmul(
                                vps, lhsT=xts[di][:, sc * 128:(sc + 1) * 128],
                                rhs=w_all[2][:, di, :],
                                start=(di == 0), stop=(di == NDIN - 1))
                        g = sg * (SG // 128) + sc
                        with tc.high_priority():
                            nc.scalar.copy(vT_sb[:, g * DHC:(g + 1) * DHC], vps)
                    for tens in (q_sb, k_sb):
                        for h in range(HC):
                            rope_half(b, tens, h, (sg % 2) * SG)

            # ---- attention + out-projection (pipelined, woven) ----
            def outproj_block(b, qw, a_pair, st):
                """Emit out-proj for one 128-row seq block of window (b, qw)
                (all 2048 cols). Called from inside the NEXT group's chunk
                loop so these ready-to-run matmuls fill the PE's exp-latency
                gaps. Both column-tiles share one LDWEIGHTS per head (4
                matmuls per stationary)."""
                ops = [psBig.tile([128, 2, W], F32, tag="psBig",
                                  name=f"o{b}_{qw}_{st}_{t}")
                       for t in range(2)]
                for h in range(HC):
                    for t in range(2):
                        for j in range(2):
                            dg = t * 2 + j
                            nc.tensor.matmul(
                                ops[t][:, j, :],
                                lhsT=a_pair[:, h, st * 128:(st + 1) * 128],
                                rhs=woT_sb[h][:, dg * W:(dg + 1) * W],
                                start=(h == 0), stop=(h == HC - 1))
                ob = obp.tile([128, 4, W], BF16, tag="ob",
                              name=f"ob{b}_{qw}_{st}")
                e0, e1 = ((nc.scalar, nc.vector) if st % 2 == 0
                          else (nc.vector, nc.scalar))
                for t in range(2):
                    eng = e0 if t == 0 else e1
                    if eng is nc.scalar:
                        nc.scalar.copy(ob[:, 2 * t:2 * t + 2, :], ops[t])
                    else:
                        nc.vector.tensor_copy(ob[:, 2 * t:2 * t + 2, :], ops[t])
                r0 = b * S + qw * W + st * 128
                nc.sync.dma_start(
                    out=out_d[r0:r0 + 128, :],
                    in_=ob.rearrange("p a b -> p (a b)"))

            pending = None
            for b in range(B):
                for qw in range(NQW):
                    active = [c for c in range(NKC) if cls[qw][c][0] != SKIP]
                    if not active:
                        continue
                    qc = qw * W
                    nact = len(active)
                    att = psAtt.tile([128, 2, W], F32, tag="psA",
                                     name=f"att{b}_{qw}")
                    acc = accp.tile([128, 2, W], BF16, tag="acc",
                                    name=f"acc{b}_{qw}")
                    n_ops = 4 if pending is not None else 0
                    ops_done = 0
                    for ci, c in enumerate(active):
                        kind, off, pm_id, p0, p1 = cls[qw][c]
                        if ci == 0:
                            # the first touch of the accumulator copies the
                            # full tile, so compute the full width
                            off = 0
                        sp = psBig.tile([128, 2, W], F32, tag="psBig",
                                        name=f"sc{b}_{qw}_{c}")
                        kc = c * 128
                        for h in range(HC):
                            last = (kind != MASKED)
                            nc.tensor.matmul(sp[:, h, off:W],
                                             lhsT=k_sb[b][h][:, kc:kc + 128],
                                             rhs=q_sb[b][h][:, qc + off:qc + W],
                                             start=True, stop=last)
                            if kind == MASKED:
                                lo = max(p0, off)
                                nc.tensor.matmul(sp[:, h, lo:p1],
                                                 lhsT=ident,
                                                 rhs=pm_sb[pm_id][:, lo:p1],
                                                 start=False, stop=True)
                        pb = pbp.tile([128, 2, W], BF16, tag="pb",
                                      name=f"pb{b}_{qw}_{c}")
                        with tc.high_priority():
                            nc.scalar.activation(pb[:, :, off:W], sp[:, :, off:W],
                                                 mybir.ActivationFunctionType.Exp,
                                                 scale=RSQRT_HD)
                        if ci == 0:
                            nc.vector.tensor_copy(acc, pb)
                        else:
                            nc.vector.tensor_add(acc[:, :, off:W],
                                                 acc[:, :, off:W],
                                                 pb[:, :, off:W])
                        g = b * NKC + c
                        for h in range(HC):
                            nc.tensor.matmul(
                                att[:, h, off:W],
                                lhsT=vT_sb[:, g * DHC + h * 128:g * DHC + (h + 1) * 128],
                                rhs=pb[:, h, off:W],
                                start=(ci == 0), stop=(ci == nact - 1))
                        # weave the pending window's out-proj matmuls into
                        # the chunk stream. Start late: the PE executes
                        # in order, so earlier op-blocks would stall the
                        # stream waiting on the previous group's normalize
                        # chain (a_pair) instead of filling gaps.
                        sci = 4 if nact > 4 else 2
                        if ci >= sci and nact > sci:
                            want = (ci - sci + 1) * n_ops // (nact - sci)
                            while ops_done < min(want, n_ops):
                                outproj_block(*pending, ops_done)
                                ops_done += 1
                    if ops_done < n_ops:
                        for o in range(ops_done, n_ops):
                            outproj_block(*pending, o)
                        ops_done = n_ops
                    # free the att PSUM bank pair immediately (psAtt is a
                    # single buf: the next group's first AV needs it) — copy
                    # the unnormalized sums out, normalize later off-path
                    araw = app.tile([128, 2, W], BF16, tag="apair",
                                    name=f"araw{b}_{qw}")
                    with tc.high_priority():
                        nc.vector.tensor_copy(araw, att)
                    # broadcast column sums of the accumulator
                    dsm = psBig.tile([128, 2, W], F32, tag="psBig",
                                     name=f"dsm{b}_{qw}")
                    for h in range(HC):
                        nc.tensor.matmul(dsm[:, h, :], lhsT=ones,
                                         rhs=acc[:, h, :],
                                         start=True, stop=True)
                    # 1/dsm = exp(-ln(dsm)): two ACT LUT passes (exact DVE
                    # reciprocal is 8 cyc/elem; ACT Reciprocal LUT is banned)
                    lnd = rcp.tile([128, 2, W], F32, tag="rc", name=f"ln{b}_{qw}")
                    rc = rcp.tile([128, 2, W], BF16, tag="rcb", name=f"rc{b}_{qw}")
                    a_pair = app.tile([128, 2, W], BF16, tag="apair",
                                      name=f"ap{b}_{qw}")
                    with tc.high_priority():
                        nc.scalar.activation(lnd, dsm,
                                             mybir.ActivationFunctionType.Ln)
                        nc.scalar.activation(rc, lnd,
                                             mybir.ActivationFunctionType.Exp,
                                             scale=-1.0)
                    nc.vector.tensor_mul(a_pair, araw, rc)
                    pending = (b, qw, a_pair)
            if pending is not None:
                for o in range(4):
                    outproj_block(*pending, o)
    _split_multi_waits(nc)
    return nc


def _prepare(x, freqs_cos, freqs_sin, mask, wq, wk, wv, wo):
    x = np.asarray(x, dtype=np.float32)
    wq = np.asarray(wq, dtype=np.float32)
    wk = np.asarray(wk, dtype=np.float32)
    wv = np.asarray(wv, dtype=np.float32)
    wo = np.asarray(wo, dtype=np.float32)
    fc = np.asarray(freqs_cos, dtype=np.float32)
    fs = np.asarray(freqs_sin, dtype=np.float32)
    mask = np.asarray(mask, dtype=np.float32)

    xT = np.ascontiguousarray(x.reshape(BS, D).T).astype(NPBF16)

    cosT = fc.T                      # [64, S]
    sinT = fs.T
    cos_dup = np.vstack([cosT, cosT])
    sin_sgn = np.vstack([-sinT, sinT])
    trig = np.ascontiguousarray(np.hstack([cos_dup, sin_sgn])).astype(NPBF16)

    em = np.exp(mask).T              # [k, q]; exp(-inf)=0, exp(0)=1
    emaskT = np.ascontiguousarray(em).astype(NPBF16)
    # binary masks (em in {0,1}) are folded into the scores PSUM as an
    # additive -30000 pattern via one extra matmul; the kernel does not
    # support soft (non-binary) masks in this build
    cls = []
    pm_blocks = []
    pm_index = {}
    for qw in range(NQW):
        row = []
        for c in range(NKC):
            t = emaskT[c * 128:(c + 1) * 128, qw * W:(qw + 1) * W]
            tf = np.asarray(t, dtype=np.float32)
            if not t.any():
                row.append((SKIP, 0, -1, 0, 0))
            elif (t == NPBF16(1.0)).all():
                row.append((FREE, 0, -1, 0, 0))
            else:
                assert np.all((tf == 0.0) | (tf == 1.0)), \
                    "only binary (0/-inf) masks supported"
                colnz = (tf != 0).any(axis=0)
                off = int(np.argmax(colnz))  # first column with any valid key
                pat = (tf == 0.0).astype(np.float32) * NEGBIG  # [128, W]
                patnz = (pat != 0).any(axis=0)
                p0 = int(np.argmax(patnz))
                p1 = int(W - np.argmax(patnz[::-1]))
                key = pat.tobytes()
                if key not in pm_index:
                    pm_index[key] = len(pm_blocks)
                    pm_blocks.append(pat.astype(NPBF16))
                row.append((MASKED, off, pm_index[key], p0, p1))
        cls.append(tuple(row))
    cls_key = tuple(cls)
    n_pm = len(pm_blocks)
    pmask = np.zeros((128, max(n_pm, 1), W), dtype=NPBF16)
    for e, blk in enumerate(pm_blocks):
        pmask[:, e, :] = blk

    # deinterleave perm: even dims then odd dims, per head
    ridx = np.concatenate([np.arange(0, HD, 2), np.arange(1, HD, 2)])
    in_maps = []
    for core in range(NCORES):
        heads = [core * HC + h for h in range(HC)]
        qk_rows = np.concatenate([g * HD + ridx for g in heads])
        v_rows = np.concatenate([np.arange(g * HD, (g + 1) * HD) for g in heads])
        m = {
            "xT": xT,
            "wqT": np.ascontiguousarray(wq[qk_rows].T).astype(NPBF16),
            "wkT": np.ascontiguousarray(wk[qk_rows].T).astype(NPBF16),
            "wvT": np.ascontiguousarray(wv[v_rows].T).astype(NPBF16),
            "woT": np.ascontiguousarray(wo[:, v_rows].T).astype(NPBF16),
            "trig": trig,
            "pmask": pmask,
        }
        in_maps.append(m)
    return in_maps, cls_key, n_pm


def kernel(x, start_pos, freqs_cos, freqs_sin, mask, wq, wk, wv, wo):
    in_maps, cls_key, n_pm = _prepare(x, freqs_cos, freqs_sin, mask, wq, wk, wv, wo)
    nc = _PROGRAM_CACHE.get(cls_key)
    if nc is None:
        nc = _build(cls_key, n_pm)
        _PROGRAM_CACHE[cls_key] = nc
    res = run_bass_kernel_spmd(
        nc, in_maps, list(range(NCORES)),
        trace=bool(os.environ.get("KERNEL_TRACE")),
        tmpdir=os.environ.get("KERNEL_TRACE_DIR") or None)
    LAST_RUN[0] = res
    out = np.zeros([BS, D], np.float32)
    for r in res.results:
        out += np.asarray(r["out"], dtype=np.float32)
    return out.reshape(B, S, D)


# revision 40
# speedup vs baseline: 1.1977x; 1.1977x over previous
"""Tensor-parallel attention block (QKV + RoPE + causal attention + out-proj)
for 8 Trainium2 NeuronCores.

Sharding: heads (16) split across 8 cores, 2 heads/core. wq/wk/wv column-
sharded, wo row-sharded; each core computes a full-shape partial output (bf16)
and the host sums the 8 partials in f32.

Measured on this part: PE runs at 2.0 GHz warm / 1.0 GHz cold (HAM), and the
kernel is PE-streaming-cycle bound, so the design minimizes PE cycles and
keeps the PE stream dense:
  - softmax denominator off the PE: the DVE accumulates exp chunks into
    two interleaved bf16 accumulators; four ones-matmuls per (b,qw) group
    broadcast the column sums; reciprocal via exp(-ln(d)) on ACT (exact
    DVE reciprocal is 8 cyc/elem, ACT Reciprocal LUT is banned for
    accuracy). GpSimd is kept off elementwise work: it shares SBUF ports
    with the DVE and starves it.
  - causal masking folded into the scores PSUM as an extra accumulate
    matmul (lhsT=identity, rhs=-30000*pattern): exp then yields exact
    zeros — no elementwise mask multiply, no extra engine hop between
    exp and the AV matmul.
  - dead causal regions skipped at column granularity (scores/exp/AV only
    over q >= off per (qw, kc) tile, off derived from the real mask).
  - V projection emitted directly in [seq, hd] layout (stationary = x
    chunk): no PE transposes.
  - out-projection pipelined one group late and its matmuls woven through
    the next group's chunk loop from a shared 3-deep PSUM ring
    (psBig 6 banks + psAtt 2 banks = all 8), so PE always has independent
    work while ACT exp latency drains; the raw attention sums are copied
    out of PSUM immediately (normalization happens off the critical path)
    so the single att accumulator recycles without waiting on the
    reciprocal chain.
  - RoPE is applied per seq-half right after that half's q/k projection
    copies, so it runs on an otherwise-idle DVE during QKV matmuls and
    the attention phase never waits on it.
  - both heads packed side by side in [128, 2, 512] tiles: one ACT exp per
    chunk, one reciprocal chain per group.
  - output written bf16; out DMAs are [128, 2048] on the SP (sync) hwdge
    queue; weight loads also on SP to parallelize with the x loads on the
    gpsimd queue.
"""

import math
import os
import sys

import numpy as np
import ml_dtypes

sys.path.insert(0, "/opt/trn_rl_repo")

import concourse.bass as bass
import concourse.mybir as mybir
from concourse.tile import TileContext
from concourse.bass_utils import run_bass_kernel_spmd
from concourse.masks import make_identity

B, S, D, H = 2, 2048, 2048, 16
HD = D // H            # 128 head dim
NCORES = 8
HC = H // NCORES       # 2 heads per core
DHC = HC * HD          # 256
BS = B * S             # 4096
NDIN = D // 128        # 16 contraction chunks
W = 512                # attention q-window
NQW = S // W           # 4 q windows per batch
NKC = S // 128         # 16 k chunks per batch
SG = 1024              # qkv seq-group width
NSG = BS // SG         # 4
RSQRT_HD = 1.0 / math.sqrt(HD)
NEGBIG = -30000.0      # additive mask value; * RSQRT_HD still << exp underflow

BF16 = mybir.dt.bfloat16
F32 = mybir.dt.float32
NPBF16 = ml_dtypes.bfloat16

SKIP, FREE, MASKED = 0, 1, 2

# stash of the last BassKernelResults for the test harness (exec_time_ns etc)
LAST_RUN = [None]
_PROGRAM_CACHE = {}


def _split_multi_waits(nc):
    """Walrus codegen allows only 1 embedded sync-wait per instruction (2 for
    EventSemaphore). Tile's sem-assignment can emit more; hoist the excess into
    standalone InstEventSemaphore waits on the same engine, just before."""
    n = 0
    for blk in nc.m.functions[0].blocks:
        out = []
        for inst in blk.instructions:
            si = getattr(inst, "sync_info", None)
            cap = 2 if isinstance(inst, mybir.InstEventSemaphore) else 1
            if si is not None and si.on_wait and len(si.on_wait) > cap:
                waits = list(si.on_wait)
                for w in waits[:-cap]:
                    n += 1
                    ev = mybir.InstEventSemaphore(
                        name=f"{inst.name}_xw{n}",
                        ins=[], outs=[],
                        sync_info=mybir.SyncInfo(on_wait=[w], on_update=[]))
                    ev.engine = inst.engine
                    out.append(ev)
                si.on_wait = waits[-cap:]
            out.append(inst)
        blk.instructions = out


def _build(cls_key, n_pm):
    """Build the per-core Bass program.

    cls_key: tuple[NQW][NKC] of (kind, off, pm_id, p0, p1); n_pm: number of
    unique deduped [128, W] additive-mask pattern blocks in `pmask`."""
    cls = [list(row) for row in cls_key]
    nc = bass.Bass()

    xT = nc.declare_dram_parameter("xT", [D, BS], BF16, isOutput=False)
    wqT = nc.declare_dram_parameter("wqT", [D, DHC], BF16, isOutput=False)
    wkT = nc.declare_dram_parameter("wkT", [D, DHC], BF16, isOutput=False)
    wvT = nc.declare_dram_parameter("wvT", [D, DHC], BF16, isOutput=False)
    woT = nc.declare_dram_parameter("woT", [DHC, D], BF16, isOutput=False)
    trig = nc.declare_dram_parameter("trig", [128, 2 * S], BF16, isOutput=False)
    pmask = nc.declare_dram_parameter("pmask", [128, max(n_pm, 1), W], BF16,
                                      isOutput=False)
    out_d = nc.declare_dram_parameter("out", [BS, D], BF16, isOutput=True)

    with TileContext(nc) as tc:
        with (
            tc.tile_pool(name="consts", bufs=1) as consts,
            tc.tile_pool(name="xt", bufs=6) as xtp,
            tc.tile_pool(name="rsw", bufs=2) as rswp,
            tc.tile_pool(name="rtmp", bufs=2) as rtp,
            tc.tile_pool(name="pb", bufs=6) as pbp,
            tc.tile_pool(name="acc", bufs=4) as accp,
            tc.tile_pool(name="apair", bufs=4) as app,
            tc.tile_pool(name="rc", bufs=2) as rcp,
            tc.tile_pool(name="ob", bufs=2) as obp,
            tc.tile_pool(name="psBig", bufs=3, space="PSUM") as psBig,
            tc.tile_pool(name="psAtt", bufs=1, space="PSUM") as psAtt,
        ):
            # ---- persistent tiles ----
            # per-batch q/k tiles: rope for batch 1 must not create (false)
            # write-dependencies against batch-0 attention reads
            q_sb = [[consts.tile([128, S], BF16, tag=f"q{b}{h}", name=f"q{b}{h}")
                     for h in range(HC)] for b in range(B)]
            k_sb = [[consts.tile([128, S], BF16, tag=f"k{b}{h}", name=f"k{b}{h}")
                     for h in range(HC)] for b in range(B)]
            vT_sb = consts.tile([128, B * NKC * DHC], BF16, tag="vT", name="vT")
            ones = consts.tile([128, 128], BF16, tag="ones", name="ones")
            nc.vector.memset(ones, 1.0)
            ident = consts.tile([128, 128], BF16, tag="ident", name="ident")
            make_identity(nc, ident)

            # warm the PE clock (HAM releases the throttle after ~3.4us of
            # sustained activity) while the first DMAs are in flight
            wu = psAtt.tile([128, 128], F32, tag="psA", name="warmup")
            for i in range(190):
                nc.tensor.matmul(wu, lhsT=ones, rhs=ones, start=True, stop=True)

            # weights resident: [128, NDIN, DHC], halves loaded separately so
            # the first projection matmuls can start sooner
            w_all = []
            for wi_, wparam in enumerate([wqT, wkT, wvT]):
                wt_ = consts.tile([128, NDIN, DHC], BF16, tag=f"wall{wi_}", name=f"wall{wi_}")
                for hf in range(2):
                    r0 = hf * (D // 2)
                    nc.sync.dma_start(
                        out=wt_[:, hf * (NDIN // 2):(hf + 1) * (NDIN // 2), :],
                        in_=wparam[r0:r0 + D // 2, :].rearrange("(n p) m -> p n m", p=128))
                w_all.append(wt_)
            trig_sb = consts.tile([128, 2 * S], BF16, tag="trig", name="trig")
            nc.sync.dma_start(out=trig_sb, in_=trig[:, :])
            woT_sb = [consts.tile([128, D], BF16, tag=f"wo{h}", name=f"wo{h}")
                      for h in range(HC)]
            for h in range(HC):
                nc.sync.dma_start(out=woT_sb[h], in_=woT[h * 128:(h + 1) * 128, :])
            pm_sb = []
            for e in range(n_pm):
                pmt = consts.tile([128, W], BF16, tag=f"pm{e}", name=f"pm{e}")
                nc.sync.dma_start(out=pmt, in_=pmask[:, e, :])
                pm_sb.append(pmt)

            def rope_half(b, tens, h, lo):
                """RoPE on positions [lo, lo+SG) of one head's q or k —
                emitted right after that seq-group's projection copies, so
                the DVE does it while the PE is still on QKV matmuls and the
                attention phase never waits on rope."""
                src = tens[b][h]
                hi = lo + SG
                sw = rswp.tile([128, SG], BF16, tag="rsw", name=f"sw{b}_{h}_{lo}")
                nc.scalar.dma_start(out=sw[0:64, :], in_=src[64:128, lo:hi])
                nc.scalar.dma_start(out=sw[64:128, :], in_=src[0:64, lo:hi])
                mcc = rtp.tile([128, SG], BF16, tag="mcc", name=f"mcc{b}_{h}_{lo}")
                nc.vector.tensor_mul(mcc, src[:, lo:hi], trig_sb[:, lo:hi])
                nc.vector.tensor_mul(sw, sw, trig_sb[:, S + lo:S + hi])
                nc.vector.tensor_add(src[:, lo:hi], mcc, sw)

            # ---- QKV projections (per batch) ----
            for b in range(B):
                for sg in range(2 * b, 2 * b + 2):
                    xts = []
                    for dj in range(4):
                        tb = xtp.tile([128, 4, SG], BF16, tag="xt", name=f"xt{sg}_{dj}")
                        nc.gpsimd.dma_start(
                            out=tb,
                            in_=xT[dj * 512:(dj + 1) * 512,
                                   sg * SG:(sg + 1) * SG].rearrange("(n p) m -> p n m", p=128))
                        for k4 in range(4):
                            xts.append(tb[:, k4, :])
                    # q, k: stationary = weight tile, moving = x
                    for ti in range(2):
                        for dh in range(HC):
                            ps = psBig.tile([128, 2, W], F32, tag="psBig",
                                            name=f"ps{sg}_{ti}_{dh}")
                            for di in range(NDIN):
                                for wi in range(2):
                                    nc.tensor.matmul(
                                        ps[:, wi, :],
                                        lhsT=w_all[ti][:, di, dh * 128:(dh + 1) * 128],
                                        rhs=xts[di][:, wi * W:(wi + 1) * W],
                                        start=(di == 0), stop=(di == NDIN - 1))
                            dst = (q_sb if ti == 0 else k_sb)[b][dh]
                            c0 = (sg % 2) * SG
                            with tc.high_priority():
                                nc.scalar.copy(
                                    dst[:, c0:c0 + SG].rearrange(
                                        "p (n m) -> p n m", n=2),
                                    ps)
                    # v: stationary = x chunk, moving = wv cols -> [seq, hd]
                    for sc in range(SG // 128):
                        vps = psBig.tile([128, DHC], F32, tag="psBig",
                                         name=f"vps{sg}_{sc}")
                        for di in range(NDIN):
                            nc.tensor.matmul(
                                vps, lhsT=xts[di][:, sc * 128:(sc + 1) * 128],
                                rhs=w_all[2][:, di, :],
                                start=(di == 0), stop=(di == NDIN - 1))
                        g = sg * (SG // 128) + sc
                        with tc.high_priority():
                            nc.scalar.copy(vT_sb[:, g * DHC:(g + 1) * DHC], vps)
                    for tens in (q_sb, k_sb):
                        for h in range(HC):
                            rope_half(b, tens, h, (sg % 2) * SG)

            # ---- attention + out-projection (pipelined, woven) ----
            def outproj_block(b, qw, a_pair, st):
                """Emit out-proj for one 128-row seq block of window (b, qw)
                (all 2048 cols). Called from inside the NEXT group's chunk
                loop so these ready-to-run matmuls fill the PE's exp-latency
                gaps. Both column-tiles share one LDWEIGHTS per head (4
                matmuls per stationary)."""
                ops = [psBig.tile([128, 2, W], F32, tag="psBig",
                                  name=f"o{b}_{qw}_{st}_{t}")
                       for t in range(2)]
                for h in range(HC):
                    for t in range(2):
                        for j in range(2):
                            dg = t * 2 + j
                            nc.tensor.matmul(
                                ops[t][:, j, :],
                                lhsT=a_pair[:, h, st * 128:(st + 1) * 128],
                                rhs=woT_sb[h][:, dg * W:(dg + 1) * W],
                                start=(h == 0), stop=(h == HC - 1))
                ob = obp.tile([128, 4, W], BF16, tag="ob",
                              name=f"ob{b}_{qw}_{st}")
                e0, e1 = ((nc.scalar, nc.vector) if st % 2 == 0
                          else (nc.vector, nc.scalar))
                for t in range(2):
                    eng = e0 if t == 0 else e1
                    if eng is nc.scalar:
                        nc.scalar.copy(ob[:, 2 * t:2 * t + 2, :], ops[t])
                    else:
                        nc.vector.tensor_copy(ob[:, 2 * t:2 * t + 2, :], ops[t])
                r0 = b * S + qw * W + st * 128
                nc.sync.dma_start(
                    out=out_d[r0:r0 + 128, :],
                    in_=ob.rearrange("p a b -> p (a b)"))

            pending = None
            for b in range(B):
                for qw in range(NQW):
                    active = [c for c in range(NKC) if cls[qw][c][0] != SKIP]
                    if not active:
                        continue
                    qc = qw * W
                    nact = len(active)
                    att = psAtt.tile([128, 2, W], F32, tag="psA",
                                     name=f"att{b}_{qw}")
                    acc = accp.tile([128, 2, W], BF16, tag="acc",
                                    name=f"acc{b}_{qw}")
                    n_ops = 4 if pending is not None else 0
                    ops_done = 0
                    for ci, c in enumerate(active):
                        kind, off, pm_id, p0, p1 = cls[qw][c]
                        if ci == 0:
                            # the first touch of the accumulator copies the
                            # full tile, so compute the full width
                            off = 0
                        sp = psBig.tile([128, 2, W], F32, tag="psBig",
                                        name=f"sc{b}_{qw}_{c}")
                        kc = c * 128
                        for h in range(HC):
                            last = (kind != MASKED)
                            nc.tensor.matmul(sp[:, h, off:W],
                                             lhsT=k_sb[b][h][:, kc:kc + 128],
                                             rhs=q_sb[b][h][:, qc + off:qc + W],
                                             start=True, stop=last)
                            if kind == MASKED:
                                lo = max(p0, off)
                                nc.tensor.matmul(sp[:, h, lo:p1],
                                                 lhsT=ident,
                                                 rhs=pm_sb[pm_id][:, lo:p1],
                                                 start=False, stop=True)
                        pb = pbp.tile([128, 2, W], BF16, tag="pb",
                                      name=f"pb{b}_{qw}_{c}")
                        with tc.high_priority():
                            nc.scalar.activation(pb[:, :, off:W], sp[:, :, off:W],
                                                 mybir.ActivationFunctionType.Exp,
                                                 scale=RSQRT_HD)
                        if ci == 0:
                            nc.vector.tensor_copy(acc, pb)
                        else:
                            nc.vector.tensor_add(acc[:, :, off:W],
                                                 acc[:, :, off:W],
                                                 pb[:, :, off:W])
                        g = b * NKC + c
                        for h in range(HC):
                            nc.tensor.matmul(
                                att[:, h, off:W],
                                lhsT=vT_sb[:, g * DHC + h * 128:g * DHC + (h + 1) * 128],
                                rhs=pb[:, h, off:W],
                                start=(ci == 0), stop=(ci == nact - 1))
                        # weave the pending window's out-proj matmuls into
                        # the chunk stream. Start late: the PE executes
                        # in order, so earlier op-blocks would stall the
                        # stream waiting on the previous group's normalize
                        # chain (a_pair) instead of filling gaps.
                        sci = 4 if nact > 4 else 2
                        if ci >= sci and nact > sci:
                            want = (ci - sci + 1) * n_ops // (nact - sci)
                            while ops_done < min(want, n_ops):
                                outproj_block(*pending, ops_done)
                                ops_done += 1
                    if ops_done < n_ops:
                        for o in range(ops_done, n_ops):
                            outproj_block(*pending, o)
                        ops_done = n_ops
                    # free the att PSUM bank pair immediately (psAtt is a
                    # single buf: the next group's first AV needs it) — copy
                    # the unnormalized sums out, normalize later off-path
                    araw = app.tile([128, 2, W], BF16, tag="apair",
                                    name=f"araw{b}_{qw}")
                    with tc.high_priority():
                        nc.vector.tensor_copy(araw, att)
                    # broadcast column sums of the accumulator
                    dsm = psBig.tile([128, 2, W], F32, tag="psBig",
                                     name=f"dsm{b}_{qw}")
                    for h in range(HC):
                        nc.tensor.matmul(dsm[:, h, :], lhsT=ones,
                                         rhs=acc[:, h, :],
                                         start=True, stop=True)
                    # 1/dsm = exp(-ln(dsm)): two ACT LUT passes (exact DVE
                    # reciprocal is 8 cyc/elem; ACT Reciprocal LUT is banned)
                    lnd = rcp.tile([128, 2, W], F32, tag="rc", name=f"ln{b}_{qw}")
                    rc = rcp.tile([128, 2, W], BF16, tag="rcb", name=f"rc{b}_{qw}")
                    a_pair = app.tile([128, 2, W], BF16, tag="apair",
                                      name=f"ap{b}_{qw}")
                    with tc.high_priority():
                        nc.scalar.activation(lnd, dsm,
                                             mybir.ActivationFunctionType.Ln)
                        nc.scalar.activation(rc, lnd,
                                             mybir.ActivationFunctionType.Exp,
                                             scale=-1.0)
                    nc.vector.tensor_mul(a_pair, araw, rc)
                    pending = (b, qw, a_pair)
            if pending is not None:
                for o in range(4):
                    outproj_block(*pending, o)
    _split_multi_waits(nc)
    return nc


def _prepare(x, freqs_cos, freqs_sin, mask, wq, wk, wv, wo):
    x = np.asarray(x, dtype=np.float32)
    wq = np.asarray(wq, dtype=np.float32)
    wk = np.asarray(wk, dtype=np.float32)
    wv = np.asarray(wv, dtype=np.float32)
    wo = np.asarray(wo, dtype=np.float32)
    fc = np.asarray(freqs_cos, dtype=np.float32)
    fs = np.asarray(freqs_sin, dtype=np.float32)
    mask = np.asarray(mask, dtype=np.float32)

    xT = np.ascontiguousarray(x.reshape(BS, D).T).astype(NPBF16)

    cosT = fc.T                      # [64, S]
    sinT = fs.T
    cos_dup = np.vstack([cosT, cosT])
    sin_sgn = np.vstack([-sinT, sinT])
    trig = np.ascontiguousarray(np.hstack([cos_dup, sin_sgn])).astype(NPBF16)

    em = np.exp(mask).T              # [k, q]; exp(-inf)=0, exp(0)=1
    emaskT = np.ascontiguousarray(em).astype(NPBF16)
    # binary masks (em in {0,1}) are folded into the scores PSUM as an
    # additive -30000 pattern via one extra matmul; the kernel does not
    # support soft (non-binary) masks in this build
    cls = []
    pm_blocks = []
    pm_index = {}
    for qw in range(NQW):
        row = []
        for c in range(NKC):
            t = emaskT[c * 128:(c + 1) * 128, qw * W:(qw + 1) * W]
            tf = np.asarray(t, dtype=np.float32)
            if not t.any():
                row.append((SKIP, 0, -1, 0, 0))
            elif (t == NPBF16(1.0)).all():
                row.append((FREE, 0, -1, 0, 0))
            else:
                assert np.all((tf == 0.0) | (tf == 1.0)), \
                    "only binary (0/-inf) masks supported"
                colnz = (tf != 0).any(axis=0)
                off = int(np.argmax(colnz))  # first column with any valid key
                pat = (tf == 0.0).astype(np.float32) * NEGBIG  # [128, W]
                patnz = (pat != 0).any(axis=0)
                p0 = int(np.argmax(patnz))
                p1 = int(W - np.argmax(patnz[::-1]))
                key = pat.tobytes()
                if key not in pm_index:
                    pm_index[key] = len(pm_blocks)
                    pm_blocks.append(pat.astype(NPBF16))
                row.append((MASKED, off, pm_index[key], p0, p1))
        cls.append(tuple(row))
    cls_key = tuple(cls)
    n_pm = len(pm_blocks)
    pmask = np.zeros((128, max(n_pm, 1), W), dtype=NPBF16)
    for e, blk in enumerate(pm_blocks):
        pmask[:, e, :] = blk

    # deinterleave perm: even dims then odd dims, per head
    ridx = np.concatenate([np.arange(0, HD, 2), np.arange(1, HD, 2)])
    in_maps = []
    for core in range(NCORES):
        heads = [core * HC + h for h in range(HC)]
        qk_rows = np.concatenate([g * HD + ridx for g in heads])
        v_rows = np.concatenate([np.arange(g * HD, (g + 1) * HD) for g in heads])
        m = {
            "xT": xT,
            "wqT": np.ascontiguousarray(wq[qk_rows].T).astype(NPBF16),
            "wkT": np.ascontiguousarray(wk[qk_rows].T).astype(NPBF16),
            "wvT": np.ascontiguousarray(wv[v_rows].T).astype(NPBF16),
            "woT": np.ascontiguousarray(wo[:, v_rows].T).astype(NPBF16),
            "trig": trig,
            "pmask": pmask,
        }
        in_maps.append(m)
    return in_maps, cls_key, n_pm


def kernel(x, start_pos, freqs_cos, freqs_sin, mask, wq, wk, wv, wo):
    in_maps, cls_key, n_pm = _prepare(x, freqs_cos, freqs_sin, mask, wq, wk, wv, wo)
    nc = _PROGRAM_CACHE.get(cls_key)
    if nc is None:
        nc = _build(cls_key, n_pm)
        _PROGRAM_CACHE[cls_key] = nc
    res = run_bass_kernel_spmd(
        nc, in_maps, list(range(NCORES)),
        trace=bool(os.environ.get("KERNEL_TRACE")),
        tmpdir=os.environ.get("KERNEL_TRACE_DIR") or None)
    LAST_RUN[0] = res
    out = np.zeros([BS, D], np.float32)
    for r in res.results:
        out += np.asarray(r["out"], dtype=np.float32)
    return out.reshape(B, S, D)
